# revision 2
# baseline (speedup 1.0000x reference)
"""Bilateral filter denoiser (5x5, sigma_s=2.0, sigma_r=0.1) on 8 Trainium2
NeuronCores — v2.  One batch element per core; full inputs in, full output out.

Math (half-offset symmetric trick):
  out = x + S1/S0,  S0 = s_c + sum_t c_t (E_t[g] + E_t[g-t]),
  S1 = sum_t c_t (m_t[g] - m_t[g-t]),   E_t = DerErf(sqrt50*dt), m_t = E_t*dt,
  c_t = s_t * sqrt(pi)/2  (DerErf(u) = 2/sqrt(pi) * exp(-u^2)).

Implementation:
  * Range weight in ONE ACT op (Derivative_Erf) — no square/exp pair; the
    per-offset spatial scale lives in the shift-matrix values.
  * 6 offsets fp8: ACT emits the weight field as fp8e4 directly, mt on Pool,
    S0/S1 accumulate via fp8 DoubleRow matmuls (lhsT [128,256] =
    [c*E2 | +-c*E_{2-di}]; rhs = fwd/bwd 512-windows of one field via a
    2-level +-dj-stride AP): both taps in 256 PE cycles.
  * 6 offsets fp16 (dt/W/mt on the DVE 2x path, plain fp16 matmuls; the two
    dj==0 offsets use combined (E2 +- E_{2-di}) matrices).
  * Groups of 2 strips; PSUM tags double-buffered (4 tags x 2 bufs = 8
    banks) so PE rolls into the next group while the epilogue drains.
  * Fields live in 6-slot parent tiles per category so one ACT/DVE/Pool op
    covers 3 offsets (amortizes the 224cyc ACT bubble); chunk order
    B1,A1,B2,A2 keeps all four engines fed.
  * Per-group batched DMAs (3-level APs) spread over the SP/Act/SWDGE
    queues; output in strip-row layout (one DMA/group), host re-gathers.
  * Epilogue: DVE reciprocal + S1*Rc read PSUM directly; +x add alternates
    DVE/Pool; s_c enters S0 via an s_c*I @ ones matmul (start=True).
"""

import numpy as np

B, C, H, W = 8, 3, 512, 512
SQ50 = float(np.sqrt(50.0))

C1 = [(0, 1), (1, -1), (1, 1), (2, 1), (2, -1), (1, 0), (2, 0)]  # fp16 path
C8 = [(1, 2), (0, 2), (2, -2), (2, 2), (1, -2)]  # fp8 path
GSZ = 2

_CACHE = {}


def _strip_plan():
    Hp = H + 4
    R = C * Hp
    strips = []
    rbase = 0
    while R - 4 - rbase > 0:
        strips.append((rbase, min(124, R - 4 - rbase)))
        rbase += 124
    return strips[-1][0] + 132, strips


def _build():
    from contextlib import ExitStack

    import concourse.bacc as bacc
    import concourse.bass as bass
    import concourse.tile as tile
    from concourse import mybir

    F32 = mybir.dt.float32
    F16 = mybir.dt.float16
    F8 = mybir.dt.float8e4
    Alu = mybir.AluOpType
    Act = mybir.ActivationFunctionType
    DR = mybir.MatmulPerfMode.DoubleRow

    Hp, Wp = H + 4, W + 4
    R = C * Hp
    Rpad, strips = _strip_plan()
    NS = Wp
    SL = GSZ * NS  # parent slot stride (cols per offset slot)

    n16 = sum(2 if dj == 0 else 3 for _, dj in C1) + 1  # +SC identity
    n8 = len(C8) * 2

    nc = bacc.Bacc(
        "TRN2",
        target_bir_lowering=False,
        debug=False,
        enable_asserts=False,
        num_devices=B,
    )
    xp = nc.dram_tensor("xp", [Rpad, Wp], F16, kind="ExternalInput").ap()
    shm = nc.dram_tensor("shm", [128, n16 * 128], F16, kind="ExternalInput").ap()
    sh8 = nc.dram_tensor("sh8", [128, n8 * 256], F8, kind="ExternalInput").ap()
    NY = 124 * len(strips)
    y = nc.dram_tensor("y", [NY, W], F32, kind="ExternalOutput").ap()

    with tile.TileContext(nc) as tc, ExitStack() as ctx:
        consts = ctx.enter_context(tc.tile_pool(name="consts", bufs=1))
        m16t = consts.tile([128, n16 * 128], F16)
        m8t = consts.tile([128, n8 * 256], F8)
        ones = consts.tile([128, W], F16)
        nc.vector.memset(ones[:], 1.0)

        i16 = {}
        k = 0
        for di, dj in C1:
            i16[(di, dj)] = k
            k += 2 if dj == 0 else 3
        SC = k
        i8 = {od: 2 * i for i, od in enumerate(C8)}

        def m16(s):
            return m16t[:, s * 128:(s + 1) * 128]

        def m8dr(s):
            return bass.AP(tensor=m8t.tensor, offset=m8t.offset + s * 256,
                           ap=[m8t.ap[0], [128, 2], [1, 128]])

        slabs = ctx.enter_context(tc.tile_pool(name="slabs", bufs=3))
        fldp = ctx.enter_context(tc.tile_pool(name="fld", bufs=2))
        accp = ctx.enter_context(tc.tile_pool(name="accum", bufs=3))
        psum = ctx.enter_context(tc.tile_pool(name="psum", bufs=2, space="PSUM"))

        # chunk schedule: (cat, [offsets], slot0); di=2 offsets all sit in
        # the late chunks so group 0 can start before its T2 slab arrives
        CHUNKS = [("b", C8[0:2], 0), ("a", C1[0:3], 0),
                  ("b", C8[2:5], 2), ("a", C1[3:7], 3)]

        groups = [strips[i:i + GSZ] for i in range(0, len(strips), GSZ)]
        prev = None  # (grp, S0, S1, T2, res, gi)

        def slotap(v, slot, col0, nh, wd):
            return bass.AP(tensor=v.tensor,
                           offset=v.offset + slot * SL + col0,
                           ap=[v.ap[0], [NS, nh], [1, wd]])

        def emit_epilogue(p):
            grp, S0, S1, T2, res, gi = p
            nh = len(grp)
            for h, (rbase, K) in enumerate(grp):
                Rc = accp.tile([128, W], F32, tag="Rc", name="Rc")
                nc.vector.reciprocal_approx_fast(out=Rc[:K, :],
                                                 in_=S0[h][:K, :])
                t = accp.tile([128, W], F32, tag="t", name="t")
                nc.vector.tensor_tensor(out=t[:K, :], in0=S1[h][:K, :],
                                        in1=Rc[:K, :], op=Alu.mult)
                nc.gpsimd.tensor_tensor(
                    out=res[:K, h * W:(h + 1) * W], in0=t[:K, :],
                    in1=T2[0:K, h * NS + 2:h * NS + 2 + W], op=Alu.add)
            Kl = grp[-1][1]
            src = bass.AP(tensor=res.tensor, offset=res.offset,
                          ap=[[res.ap[0][0], Kl], [W, nh], [1, W]])
            dst = bass.AP(tensor=y.tensor, offset=(124 * GSZ * gi) * W,
                          ap=[[W, Kl], [124 * W, nh], [1, W]])
            nc.sync.dma_start(out=dst, in_=src)

        NA, NB = len(C1), len(C8)
        for gi, grp in enumerate(groups):
            nh = len(grp)
            rbase0 = grp[0][0]
            T = [slabs.tile([128, GSZ * NS], F16, tag=f"T{v}", name=f"T{v}")
                 for v in range(3)]

            def slab_dma(v, eng):
                src = bass.AP(
                    tensor=xp.tensor,
                    offset=(rbase0 + v) * Wp,
                    ap=[[Wp, 128], [124 * Wp, nh], [1, Wp]])
                dst = bass.AP(
                    tensor=T[v].tensor, offset=T[v].offset,
                    ap=[T[v].ap[0], [NS, nh], [1, Wp]])
                eng.dma_start(out=dst, in_=src)

            slab_dma(0, nc.sync)
            slab_dma(1, nc.scalar)
            slab_dma(2, nc.sync)
            if gi == 0:
                nc.sync.dma_start(out=m16t[:, SC * 128:],
                                  in_=shm[:, SC * 128:])
                nc.sync.dma_start(out=m16t[:, :SC * 128],
                                  in_=shm[:, :SC * 128])
                nc.scalar.dma_start(out=m8t[:], in_=sh8[:, :])

            dtA = fldp.tile([128, NA * SL], F16, tag="dtA", name="dtA")
            dtB = fldp.tile([128, NB * SL], F16, tag="dtB", name="dtB")
            WA = fldp.tile([128, NA * SL], F16, tag="WA", name="WA")
            WB = fldp.tile([128, NB * SL], F8, tag="WB", name="WB")
            mA = fldp.tile([128, NA * SL], F16, tag="mA", name="mA")
            mB = fldp.tile([128, NB * SL], F8, tag="mB", name="mB")
            res = accp.tile([128, GSZ * W], F32, tag="res", name="res")

            S0 = [psum.tile([128, W], F32, tag=f"S0p{h}", name=f"S0p{h}")
                  for h in range(nh)]
            S1 = [psum.tile([128, W], F32, tag=f"S1p{h}", name=f"S1p{h}")
                  for h in range(nh)]

            def emit_fields(cat, offs, slot0):
                dt, Wt, mt = (dtB, WB, mB) if cat == "b" else (dtA, WA, mA)
                for si, (di, dj) in enumerate(offs):
                    cl = min(0, -dj)
                    wd = max(W, W - dj) - cl
                    nc.vector.tensor_tensor(
                        out=slotap(dt, slot0 + si, cl + 2, nh, wd),
                        in0=bass.AP(
                            tensor=T[di].tensor,
                            offset=T[di].offset + cl + dj + 2,
                            ap=[T[di].ap[0], [NS, nh], [1, wd]]),
                        in1=bass.AP(
                            tensor=T[0].tensor, offset=T[0].offset + cl + 2,
                            ap=[T[0].ap[0], [NS, nh], [1, wd]]),
                        op=Alu.subtract)
                ns = len(offs)
                cr = bass.AP(tensor=dt.tensor, offset=dt.offset + slot0 * SL,
                             ap=[dt.ap[0], [SL, ns], [1, nh * NS]])
                cw = bass.AP(tensor=Wt.tensor, offset=Wt.offset + slot0 * SL,
                             ap=[Wt.ap[0], [SL, ns], [1, nh * NS]])
                nc.scalar.activation(cw, cr, Act.Derivative_Erf, scale=SQ50)
                cm = bass.AP(tensor=mt.tensor, offset=mt.offset + slot0 * SL,
                             ap=[mt.ap[0], [SL, ns], [1, nh * NS]])
                eng = nc.gpsimd if cat == "b" else nc.vector
                eng.tensor_tensor(out=cm, in0=cw, in1=cr, op=Alu.mult)

            def emit_mms(cat, offs, slot0, first, lastc):
                dt, Wt, mt = (dtB, WB, mB) if cat == "b" else (dtA, WA, mA)
                for h in range(nh):
                    for si, (di, dj) in enumerate(offs):
                        slot = slot0 + si
                        base = slot * SL + h * NS
                        fw = base + 2
                        bw = base + 2 - dj
                        st = first and si == 0
                        stop = lastc and si == len(offs) - 1
                        if cat == "b":
                            ii = i8[(di, dj)]
                            rhsW = bass.AP(tensor=Wt.tensor,
                                           offset=Wt.offset + fw,
                                           ap=[Wt.ap[0], [-dj, 2], [1, W]])
                            rhsM = bass.AP(tensor=mt.tensor,
                                           offset=mt.offset + fw,
                                           ap=[mt.ap[0], [-dj, 2], [1, W]])
                            nc.tensor.matmul(S0[h][:, :], m8dr(ii), rhsW,
                                             start=False, stop=stop,
                                             perf_mode=DR)
                            nc.tensor.matmul(S1[h][:, :], m8dr(ii + 1), rhsM,
                                             start=st, stop=stop,
                                             perf_mode=DR)
                        elif dj == 0:
                            ii = i16[(di, dj)]
                            nc.tensor.matmul(
                                S0[h][:, :], m16(ii),
                                Wt[:, fw:fw + W], start=False, stop=stop)
                            nc.tensor.matmul(
                                S1[h][:, :], m16(ii + 1),
                                mt[:, fw:fw + W], start=st, stop=stop)
                        else:
                            ii = i16[(di, dj)]
                            nc.tensor.matmul(
                                S0[h][:, :], m16(ii),
                                Wt[:, fw:fw + W], start=False, stop=False)
                            nc.tensor.matmul(
                                S0[h][:, :], m16(ii + 1),
                                Wt[:, bw:bw + W], start=False, stop=stop)
                            nc.tensor.matmul(
                                S1[h][:, :], m16(ii),
                                mt[:, fw:fw + W], start=st, stop=False)
                            nc.tensor.matmul(
                                S1[h][:, :], m16(ii + 2),
                                mt[:, bw:bw + W], start=False, stop=stop)

            # s_c * I @ ones opens S0 for each strip; PE-ready immediately
            for h in range(nh):
                nc.tensor.matmul(S0[h][:, :], m16(SC), ones[:, :],
                                 start=True, stop=False)
            # fields B then A; matmuls A (fast DVE mt) then B (slow Pool mt)
            emit_fields(CHUNKS[0][0], CHUNKS[0][1], CHUNKS[0][2])
            emit_fields(CHUNKS[1][0], CHUNKS[1][1], CHUNKS[1][2])
            emit_mms(CHUNKS[1][0], CHUNKS[1][1], CHUNKS[1][2], True, False)
            emit_mms(CHUNKS[0][0], CHUNKS[0][1], CHUNKS[0][2], False, False)
            if prev is not None:
                emit_epilogue(prev)
            emit_fields(CHUNKS[2][0], CHUNKS[2][1], CHUNKS[2][2])
            emit_fields(CHUNKS[3][0], CHUNKS[3][1], CHUNKS[3][2])
            emit_mms(CHUNKS[3][0], CHUNKS[3][1], CHUNKS[3][2], False, False)
            emit_mms(CHUNKS[2][0], CHUNKS[2][1], CHUNKS[2][2], False, True)

            prev = (grp, S0, S1, T[2], res, gi)

        emit_epilogue(prev)

    nc.compile()
    return nc


def _get_module():
    if "nc" not in _CACHE:
        _CACHE["nc"] = _build()
    return _CACHE["nc"]


def _pack_core(xc):
    """xc [C,H,W] f32 -> reflect-padded fp16 [Rpad, W+4]."""
    Rpad, _ = _strip_plan()
    xpad = np.pad(xc, ((0, 0), (2, 2), (2, 2)), mode="reflect")
    flat = xpad.reshape(C * (H + 4), W + 4)
    extra = Rpad - flat.shape[0]
    if extra > 0:
        flat = np.concatenate([flat, np.repeat(flat[-1:], extra, axis=0)],
                              axis=0)
    return np.ascontiguousarray(flat, dtype=np.float16)


def _pack_mats(spatial):
    import ml_dtypes

    E = [np.eye(128, 128, k=-s, dtype=np.float32) for s in range(3)]
    CS = np.sqrt(np.pi) / 2.0
    m16, m8 = [], []
    for di, dj in C1:
        c = float(spatial[2 + di, 2 + dj]) * CS
        if dj == 0:
            m16.append((c * (E[2] + E[2 - di])).astype(np.float16))
            m16.append((c * (E[2] - E[2 - di])).astype(np.float16))
        else:
            m16.append((c * E[2]).astype(np.float16))
            m16.append((c * E[2 - di]).astype(np.float16))
            m16.append((-c * E[2 - di]).astype(np.float16))
    m16.append(np.eye(128, dtype=np.float16) * np.float16(float(spatial[2, 2])))
    for di, dj in C8:
        c = float(spatial[2 + di, 2 + dj]) * CS
        m8.append(np.concatenate([c * E[2], c * E[2 - di]], axis=1)
                  .astype(ml_dtypes.float8_e4m3))
        m8.append(np.concatenate([c * E[2], -c * E[2 - di]], axis=1)
                  .astype(ml_dtypes.float8_e4m3))
    return (np.concatenate(m16, axis=1),
            np.concatenate(m8, axis=1))


def kernel(x, spatial, _trace=False):
    from concourse.bass_utils import run_bass_kernel_spmd

    x = np.asarray(x, dtype=np.float32)
    spatial = np.asarray(spatial, dtype=np.float32)
    assert x.shape == (B, C, H, W) and spatial.shape == (5, 5)
    assert np.allclose(spatial, spatial[::-1, ::-1], rtol=1e-5), \
        "kernel assumes point-symmetric spatial weights"

    shm, sh8 = _pack_mats(spatial)
    nc = _get_module()
    in_maps = [{"xp": _pack_core(x[b]), "shm": shm, "sh8": sh8}
               for b in range(B)]
    res = run_bass_kernel_spmd(nc, in_maps, core_ids=list(range(B)),
                               trace=_trace)
    gidx = (np.arange(C)[:, None] * (H + 4) + np.arange(H)[None, :]).ravel()
    out = np.stack([
        res.results[b]["y"][gidx].reshape(C, H, W) for b in range(B)])
    if _trace:
        return out.astype(np.float32), res
    return out.astype(np.float32)


# revision 3
# speedup vs baseline: 1.0017x; 1.0017x over previous
"""Bilateral filter denoiser (5x5, sigma_s=2.0, sigma_r=0.1) on 8 Trainium2
NeuronCores — v2.  One batch element per core; full inputs in, full output out.

Math (half-offset symmetric trick):
  out = x + S1/S0,  S0 = s_c + sum_t c_t (E_t[g] + E_t[g-t]),
  S1 = sum_t c_t (m_t[g] - m_t[g-t]),   E_t = DerErf(sqrt50*dt), m_t = E_t*dt,
  c_t = s_t * sqrt(pi)/2  (DerErf(u) = 2/sqrt(pi) * exp(-u^2)).

Implementation:
  * Range weight in ONE ACT op (Derivative_Erf) — no square/exp pair; the
    per-offset spatial scale lives in the shift-matrix values.
  * 6 offsets fp8: ACT emits the weight field as fp8e4 directly, mt on Pool,
    S0/S1 accumulate via fp8 DoubleRow matmuls (lhsT [128,256] =
    [c*E2 | +-c*E_{2-di}]; rhs = fwd/bwd 512-windows of one field via a
    2-level +-dj-stride AP): both taps in 256 PE cycles.
  * 6 offsets fp16 (dt/W/mt on the DVE 2x path, plain fp16 matmuls; the two
    dj==0 offsets use combined (E2 +- E_{2-di}) matrices).
  * Groups of 2 strips; PSUM tags double-buffered (4 tags x 2 bufs = 8
    banks) so PE rolls into the next group while the epilogue drains.
  * Fields live in 6-slot parent tiles per category so one ACT/DVE/Pool op
    covers 3 offsets (amortizes the 224cyc ACT bubble); chunk order
    B1,A1,B2,A2 keeps all four engines fed.
  * Per-group batched DMAs (3-level APs) spread over the SP/Act/SWDGE
    queues; output in strip-row layout (one DMA/group), host re-gathers.
  * Epilogue: DVE reciprocal + S1*Rc read PSUM directly; +x add alternates
    DVE/Pool; s_c enters S0 via an s_c*I @ ones matmul (start=True).
"""

import numpy as np

B, C, H, W = 8, 3, 512, 512
SQ50 = float(np.sqrt(50.0))

C1 = [(0, 1), (1, -1), (1, 1), (2, 1), (2, -1), (1, 0), (2, 0)]  # fp16 path
C8 = [(1, 2), (0, 2), (2, -2), (2, 2), (1, -2)]  # fp8 path
GSZ = 2

_CACHE = {}


def _strip_plan():
    Hp = H + 4
    R = C * Hp
    strips = []
    rbase = 0
    while R - 4 - rbase > 0:
        strips.append((rbase, min(124, R - 4 - rbase)))
        rbase += 124
    return strips[-1][0] + 132, strips


def _build():
    from contextlib import ExitStack

    import concourse.bacc as bacc
    import concourse.bass as bass
    import concourse.tile as tile
    from concourse import mybir

    F32 = mybir.dt.float32
    F16 = mybir.dt.float16
    F8 = mybir.dt.float8e4
    Alu = mybir.AluOpType
    Act = mybir.ActivationFunctionType
    DR = mybir.MatmulPerfMode.DoubleRow

    Hp, Wp = H + 4, W + 4
    R = C * Hp
    Rpad, strips = _strip_plan()
    NS = Wp
    SL = GSZ * NS  # parent slot stride (cols per offset slot)

    n16 = sum(2 if dj == 0 else 3 for _, dj in C1) + 1  # +SC identity
    n8 = len(C8) * 2

    nc = bacc.Bacc(
        "TRN2",
        target_bir_lowering=False,
        debug=False,
        enable_asserts=False,
        num_devices=B,
    )
    xp = nc.dram_tensor("xp", [Rpad, Wp], F16, kind="ExternalInput").ap()
    shm = nc.dram_tensor("shm", [128, n16 * 128], F16, kind="ExternalInput").ap()
    sh8 = nc.dram_tensor("sh8", [128, n8 * 256], F8, kind="ExternalInput").ap()
    NY = 124 * len(strips)
    y = nc.dram_tensor("y", [NY, W], F32, kind="ExternalOutput").ap()

    with tile.TileContext(nc) as tc, ExitStack() as ctx:
        consts = ctx.enter_context(tc.tile_pool(name="consts", bufs=1))
        m16t = consts.tile([128, n16 * 128], F16)
        m8t = consts.tile([128, n8 * 256], F8)
        ones = consts.tile([128, W], F16)
        nc.vector.memset(ones[:], 1.0)

        i16 = {}
        k = 0
        for di, dj in C1:
            i16[(di, dj)] = k
            k += 2 if dj == 0 else 3
        SC = k
        i8 = {od: 2 * i for i, od in enumerate(C8)}

        def m16(s):
            return m16t[:, s * 128:(s + 1) * 128]

        def m8dr(s):
            return bass.AP(tensor=m8t.tensor, offset=m8t.offset + s * 256,
                           ap=[m8t.ap[0], [128, 2], [1, 128]])

        slabs = ctx.enter_context(tc.tile_pool(name="slabs", bufs=3))
        fldp = ctx.enter_context(tc.tile_pool(name="fld", bufs=2))
        accp = ctx.enter_context(tc.tile_pool(name="accum", bufs=4))
        psum = ctx.enter_context(tc.tile_pool(name="psum", bufs=2, space="PSUM"))

        # chunk schedule: (cat, [offsets], slot0); di=2 offsets all sit in
        # the late chunks so group 0 can start before its T2 slab arrives
        CHUNKS = [("b", C8[0:2], 0), ("a", C1[0:3], 0),
                  ("b", C8[2:5], 2), ("a", C1[3:7], 3)]

        groups = [strips[i:i + GSZ] for i in range(0, len(strips), GSZ)]
        groups = [(g, i) for i, g in enumerate(groups)]
        prev = None  # (grp, S0, S1, T2, res, gi)

        def slotap(v, slot, col0, nh, wd):
            return bass.AP(tensor=v.tensor,
                           offset=v.offset + slot * SL + col0,
                           ap=[v.ap[0], [NS, nh], [1, wd]])

        def emit_epilogue(p):
            grp, S0, S1, T2, res, gi = p
            nh = len(grp)
            for h, (rbase, K) in enumerate(grp):
                Rc = accp.tile([128, W], F32, tag="Rc", name="Rc")
                nc.vector.reciprocal_approx_fast(out=Rc[:K, :],
                                                 in_=S0[h][:K, :])
                t = accp.tile([128, W], F32, tag="t", name="t")
                nc.vector.tensor_tensor(out=t[:K, :], in0=S1[h][:K, :],
                                        in1=Rc[:K, :], op=Alu.mult)
                nc.gpsimd.tensor_tensor(
                    out=res[:K, h * W:(h + 1) * W], in0=t[:K, :],
                    in1=T2[0:K, h * NS + 2:h * NS + 2 + W], op=Alu.add)
            Kl = grp[-1][1]
            src = bass.AP(tensor=res.tensor, offset=res.offset,
                          ap=[[res.ap[0][0], Kl], [W, nh], [1, W]])
            dst = bass.AP(tensor=y.tensor, offset=(124 * GSZ * gi) * W,
                          ap=[[W, Kl], [124 * W, nh], [1, W]])
            nc.sync.dma_start(out=dst, in_=src)

        NA, NB = len(C1), len(C8)
        for grp, gi in groups:
            nh = len(grp)
            rbase0 = grp[0][0]
            T = [slabs.tile([128, GSZ * NS], F16, tag=f"T{v}", name=f"T{v}")
                 for v in range(3)]

            def slab_dma(v, eng):
                src = bass.AP(
                    tensor=xp.tensor,
                    offset=(rbase0 + v) * Wp,
                    ap=[[Wp, 128], [124 * Wp, nh], [1, Wp]])
                dst = bass.AP(
                    tensor=T[v].tensor, offset=T[v].offset,
                    ap=[T[v].ap[0], [NS, nh], [1, Wp]])
                eng.dma_start(out=dst, in_=src)

            slab_dma(0, nc.sync)
            slab_dma(1, nc.scalar)
            slab_dma(2, nc.sync)
            if gi == 0:
                nc.sync.dma_start(out=m16t[:, SC * 128:],
                                  in_=shm[:, SC * 128:])
                nc.sync.dma_start(out=m16t[:, :SC * 128],
                                  in_=shm[:, :SC * 128])
                nc.scalar.dma_start(out=m8t[:], in_=sh8[:, :])

            dtA = fldp.tile([128, NA * SL], F16, tag="dtA", name="dtA")
            dtB = fldp.tile([128, NB * SL], F16, tag="dtB", name="dtB")
            WA = fldp.tile([128, NA * SL], F16, tag="WA", name="WA")
            WB = fldp.tile([128, NB * SL], F8, tag="WB", name="WB")
            mA = fldp.tile([128, NA * SL], F16, tag="mA", name="mA")
            mB = fldp.tile([128, NB * SL], F8, tag="mB", name="mB")
            res = accp.tile([128, GSZ * W], F32, tag="res", name="res")

            S0 = [psum.tile([128, W], F32, tag=f"S0p{h}", name=f"S0p{h}")
                  for h in range(nh)]
            S1 = [psum.tile([128, W], F32, tag=f"S1p{h}", name=f"S1p{h}")
                  for h in range(nh)]

            def emit_fields(cat, offs, slot0):
                dt, Wt, mt = (dtB, WB, mB) if cat == "b" else (dtA, WA, mA)
                for si, (di, dj) in enumerate(offs):
                    cl = min(0, -dj)
                    wd = max(W, W - dj) - cl
                    nc.vector.tensor_tensor(
                        out=slotap(dt, slot0 + si, cl + 2, nh, wd),
                        in0=bass.AP(
                            tensor=T[di].tensor,
                            offset=T[di].offset + cl + dj + 2,
                            ap=[T[di].ap[0], [NS, nh], [1, wd]]),
                        in1=bass.AP(
                            tensor=T[0].tensor, offset=T[0].offset + cl + 2,
                            ap=[T[0].ap[0], [NS, nh], [1, wd]]),
                        op=Alu.subtract)
                ns = len(offs)
                cr = bass.AP(tensor=dt.tensor, offset=dt.offset + slot0 * SL,
                             ap=[dt.ap[0], [SL, ns], [1, nh * NS]])
                cw = bass.AP(tensor=Wt.tensor, offset=Wt.offset + slot0 * SL,
                             ap=[Wt.ap[0], [SL, ns], [1, nh * NS]])
                nc.scalar.activation(cw, cr, Act.Derivative_Erf, scale=SQ50)
                cm = bass.AP(tensor=mt.tensor, offset=mt.offset + slot0 * SL,
                             ap=[mt.ap[0], [SL, ns], [1, nh * NS]])
                eng = nc.gpsimd if cat == "b" else nc.vector
                eng.tensor_tensor(out=cm, in0=cw, in1=cr, op=Alu.mult)

            def emit_mms(cat, offs, slot0, first, lastc):
                dt, Wt, mt = (dtB, WB, mB) if cat == "b" else (dtA, WA, mA)
                for h in range(nh):
                    for si, (di, dj) in enumerate(offs):
                        slot = slot0 + si
                        base = slot * SL + h * NS
                        fw = base + 2
                        bw = base + 2 - dj
                        st = first and si == 0
                        stop = lastc and si == len(offs) - 1
                        if cat == "b":
                            ii = i8[(di, dj)]
                            rhsW = bass.AP(tensor=Wt.tensor,
                                           offset=Wt.offset + fw,
                                           ap=[Wt.ap[0], [-dj, 2], [1, W]])
                            rhsM = bass.AP(tensor=mt.tensor,
                                           offset=mt.offset + fw,
                                           ap=[mt.ap[0], [-dj, 2], [1, W]])
                            nc.tensor.matmul(S0[h][:, :], m8dr(ii), rhsW,
                                             start=False, stop=stop,
                                             perf_mode=DR)
                            nc.tensor.matmul(S1[h][:, :], m8dr(ii + 1), rhsM,
                                             start=st, stop=stop,
                                             perf_mode=DR)
                        elif dj == 0:
                            ii = i16[(di, dj)]
                            nc.tensor.matmul(
                                S0[h][:, :], m16(ii),
                                Wt[:, fw:fw + W], start=False, stop=stop)
                            nc.tensor.matmul(
                                S1[h][:, :], m16(ii + 1),
                                mt[:, fw:fw + W], start=st, stop=stop)
                        else:
                            ii = i16[(di, dj)]
                            nc.tensor.matmul(
                                S0[h][:, :], m16(ii),
                                Wt[:, fw:fw + W], start=False, stop=False)
                            nc.tensor.matmul(
                                S0[h][:, :], m16(ii + 1),
                                Wt[:, bw:bw + W], start=False, stop=stop)
                            nc.tensor.matmul(
                                S1[h][:, :], m16(ii),
                                mt[:, fw:fw + W], start=st, stop=False)
                            nc.tensor.matmul(
                                S1[h][:, :], m16(ii + 2),
                                mt[:, bw:bw + W], start=False, stop=stop)

            # s_c * I @ ones opens S0 for each strip; PE-ready immediately
            for h in range(nh):
                nc.tensor.matmul(S0[h][:, :], m16(SC), ones[:, :],
                                 start=True, stop=False)
            # fields B then A; matmuls A (fast DVE mt) then B (slow Pool mt)
            emit_fields(CHUNKS[0][0], CHUNKS[0][1], CHUNKS[0][2])
            emit_fields(CHUNKS[1][0], CHUNKS[1][1], CHUNKS[1][2])
            emit_mms(CHUNKS[1][0], CHUNKS[1][1], CHUNKS[1][2], True, False)
            emit_mms(CHUNKS[0][0], CHUNKS[0][1], CHUNKS[0][2], False, False)
            if prev is not None:
                emit_epilogue(prev)
            emit_fields(CHUNKS[2][0], CHUNKS[2][1], CHUNKS[2][2])
            emit_fields(CHUNKS[3][0], CHUNKS[3][1], CHUNKS[3][2])
            emit_mms(CHUNKS[3][0], CHUNKS[3][1], CHUNKS[3][2], False, False)
            emit_mms(CHUNKS[2][0], CHUNKS[2][1], CHUNKS[2][2], False, True)

            prev = (grp, S0, S1, T[2], res, gi)

        emit_epilogue(prev)

    nc.compile()
    return nc


def _get_module():
    if "nc" not in _CACHE:
        _CACHE["nc"] = _build()
    return _CACHE["nc"]


def _pack_core(xc):
    """xc [C,H,W] f32 -> reflect-padded fp16 [Rpad, W+4]."""
    Rpad, _ = _strip_plan()
    xpad = np.pad(xc, ((0, 0), (2, 2), (2, 2)), mode="reflect")
    flat = xpad.reshape(C * (H + 4), W + 4)
    extra = Rpad - flat.shape[0]
    if extra > 0:
        flat = np.concatenate([flat, np.repeat(flat[-1:], extra, axis=0)],
                              axis=0)
    return np.ascontiguousarray(flat, dtype=np.float16)


def _pack_mats(spatial):
    import ml_dtypes

    E = [np.eye(128, 128, k=-s, dtype=np.float32) for s in range(3)]
    CS = np.sqrt(np.pi) / 2.0
    m16, m8 = [], []
    for di, dj in C1:
        c = float(spatial[2 + di, 2 + dj]) * CS
        if dj == 0:
            m16.append((c * (E[2] + E[2 - di])).astype(np.float16))
            m16.append((c * (E[2] - E[2 - di])).astype(np.float16))
        else:
            m16.append((c * E[2]).astype(np.float16))
            m16.append((c * E[2 - di]).astype(np.float16))
            m16.append((-c * E[2 - di]).astype(np.float16))
    m16.append(np.eye(128, dtype=np.float16) * np.float16(float(spatial[2, 2])))
    for di, dj in C8:
        c = float(spatial[2 + di, 2 + dj]) * CS
        m8.append(np.concatenate([c * E[2], c * E[2 - di]], axis=1)
                  .astype(ml_dtypes.float8_e4m3))
        m8.append(np.concatenate([c * E[2], -c * E[2 - di]], axis=1)
                  .astype(ml_dtypes.float8_e4m3))
    return (np.concatenate(m16, axis=1),
            np.concatenate(m8, axis=1))


def kernel(x, spatial, _trace=False):
    from concourse.bass_utils import run_bass_kernel_spmd

    x = np.asarray(x, dtype=np.float32)
    spatial = np.asarray(spatial, dtype=np.float32)
    assert x.shape == (B, C, H, W) and spatial.shape == (5, 5)
    assert np.allclose(spatial, spatial[::-1, ::-1], rtol=1e-5), \
        "kernel assumes point-symmetric spatial weights"

    shm, sh8 = _pack_mats(spatial)
    nc = _get_module()
    in_maps = [{"xp": _pack_core(x[b]), "shm": shm, "sh8": sh8}
               for b in range(B)]
    res = run_bass_kernel_spmd(nc, in_maps, core_ids=list(range(B)),
                               trace=_trace)
    gidx = (np.arange(C)[:, None] * (H + 4) + np.arange(H)[None, :]).ravel()
    out = np.stack([
        res.results[b]["y"][gidx].reshape(C, H, W) for b in range(B)])
    if _trace:
        return out.astype(np.float32), res
    return out.astype(np.float32)
